# revision 1
# baseline (speedup 1.0000x reference)
"""MoE MLP (top-2 routing, 8 experts) on 8 Trainium2 NeuronCores.

Strategy (expert-parallel, per the sharding hint): each core owns one
expert's weights. The router (a [8,1024] matmul + softmax + top-2 —
0.05% of total FLOPs) runs on the host, which doubles as the dispatch
step: tokens are gathered per selected expert and shipped to that
expert's core, replacing the all-to-all. Each core runs a fused
gelu-MLP Bass kernel over its routed tokens:

    yT = w ⊙ (W_out^T @ gelu(W_in^T @ xT + b_in) + b_out)

in a transposed layout (tokens along the free axis) so both matmuls
keep the *weights* stationary on the PE array and no on-chip
transposes are needed anywhere. W_out stays resident in SBUF; W_in
streams once per token chunk. The host scatter-adds the per-expert
results back into the full [B,S,D] output.

Matmuls run in fp16 (same PE throughput as bf16 — 4x fp32 — but 8x
finer mantissa; measured end-to-end error vs the fp32 reference is
~4e-4 scale-relative). Set MOE_PREC=fp32 to force full fp32 matmuls.
"""

import contextlib
import ctypes
import os
import sys
import types
from contextlib import ExitStack

import numpy as np

import concourse.bass as bass
import concourse.mybir as mybir
import concourse.tile as tile
from concourse import bacc
from concourse.bass_utils import run_bass_kernel_spmd


def _install_ntff_hook():
    """Provide antenv.axon_hooks (absent in this image) so BASS_TRACE=1
    can capture NTFF profiles through the axon PJRT .so. No-op if the
    module already exists or the .so/symbols are unavailable."""
    try:
        from antenv.axon_hooks import get_axon_ntff_profile_hook  # noqa: F401
        return
    except ImportError:
        pass
    so_path = "/opt/axon/libaxon_pjrt.so"
    if not os.path.exists(so_path):
        return
    try:
        lib = ctypes.CDLL(so_path)
    except OSError:
        return
    if not hasattr(lib, "axon_start_nrt_profile"):
        return
    lib.axon_start_nrt_profile.argtypes = [
        ctypes.POINTER(ctypes.c_int64), ctypes.c_size_t]
    lib.axon_start_nrt_profile.restype = ctypes.c_int64
    lib.axon_stop_nrt_profile.argtypes = [ctypes.c_char_p]
    lib.axon_stop_nrt_profile.restype = ctypes.c_int64

    @contextlib.contextmanager
    def _hook(output_dir, device_ids):
        import jax
        jax.devices()  # force PJRT init so the .so's client exists
        if device_ids:
            ids = (ctypes.c_int64 * len(device_ids))(*device_ids)
            rc = lib.axon_start_nrt_profile(ids, len(device_ids))
        else:
            rc = lib.axon_start_nrt_profile(None, 0)
        if rc != 0:
            raise RuntimeError(f"axon_start_nrt_profile rc={rc}")
        try:
            yield
        finally:
            n = lib.axon_stop_nrt_profile(str(output_dir).encode())
            print(f"ntff profile: {n} file(s) -> {output_dir}", file=sys.stderr)

    import antenv
    mod = types.ModuleType("antenv.axon_hooks")
    mod.get_axon_ntff_profile_hook = lambda: _hook
    mod.set_axon_ntff_profile_hook = lambda h: None
    sys.modules["antenv.axon_hooks"] = mod
    antenv.axon_hooks = mod

B, S, D, F, E = 4, 2048, 1024, 4096, 8
T = B * S
TOP_K = 2
NCORES = 8
P = 128
ND, NF = D // P, F // P  # 8, 32

# test.py pokes these for profiling info
LAST_RESULT = None

_cache = {}


def _chunk_list(C):
    """Token chunks (PSUM free-dim <= 512, multiples of 128).

    Chunks below 256 run LDWEIGHTS-bound on the PE (weight load ~60ns
    vs a 53ns N=128 matmul), so a short tail is split off the previous
    512 chunk into two >=256 pieces instead.
    """
    chunks = [512] * (C // 512)
    rem = C % 512
    if rem:
        if rem < 256 and chunks:
            total = 512 + rem
            a = ((total // 2 + 127) // 128) * 128
            chunks[-1] = a
            chunks.append(total - a)
        else:
            chunks.append(rem)
    return chunks


def _build_bass(C, prec):
    dt = mybir.dt
    fp16_path = prec != "fp32"
    io_dt = dt.float16 if fp16_path else dt.float32
    nc = bacc.Bacc("TRN2", target_bir_lowering=False, debug=False)

    xT = nc.dram_tensor("xT", [D, C], io_dt, kind="ExternalInput")
    win = nc.dram_tensor("win", [D, F], io_dt, kind="ExternalInput")
    wout = nc.dram_tensor("wout", [F, D], io_dt, kind="ExternalInput")
    bin_ = nc.dram_tensor("bin", [F], dt.float32, kind="ExternalInput")
    bout = nc.dram_tensor("bout", [D], dt.float32, kind="ExternalInput")
    wcomb = nc.dram_tensor("wcomb", [P, C], dt.float32, kind="ExternalInput")
    yT = nc.dram_tensor("yT", [D, C], dt.float32, kind="ExternalOutput")

    xT_r = xT.ap().rearrange("(dn p) c -> p dn c", p=P)
    win_r = win.ap().rearrange("(dn p) f -> p dn f", p=P)
    wout_r = wout.ap().rearrange("(fn p) d -> p fn d", p=P)
    yT_r = yT.ap().rearrange("(dn p) c -> p dn c", p=P)

    chunks = _chunk_list(C)

    with tile.TileContext(nc) as tc, ExitStack() as ctx:
        consts = ctx.enter_context(tc.tile_pool(name="consts", bufs=1))
        xpool = ctx.enter_context(tc.tile_pool(name="x", bufs=2))
        winpool = ctx.enter_context(tc.tile_pool(name="win", bufs=3))
        woutpool = ctx.enter_context(tc.tile_pool(name="wout", bufs=1))
        hpool = ctx.enter_context(tc.tile_pool(name="h", bufs=1))
        ypool = ctx.enter_context(tc.tile_pool(name="y", bufs=4))
        psum_h = ctx.enter_context(tc.tile_pool(name="ph", bufs=4, space="PSUM"))
        psum_y = ctx.enter_context(tc.tile_pool(name="py", bufs=2, space="PSUM"))

        def x_dma(ck, csl):
            x_t = xpool.tile([P, ND, ck], io_dt, tag="x")
            nc.sync.dma_start(x_t[:], xT_r[:, :, csl])
            return x_t

        def win_dma(fo):
            win_t = winpool.tile([P, ND, 512], io_dt, tag="win")
            nc.sync.dma_start(win_t[:], win_r[:, :, fo * 512:(fo + 1) * 512])
            return win_t

        # critical path for the very first matmul: x chunk 0 + W_in
        # stripe 0 go FIRST, each split across BOTH HWDGE queues (Sync +
        # Act) — a single dma_start runs ~150 GB/s, so two in parallel
        # roughly halve the time to first matmul.
        ck0 = chunks[0]
        x0_t = xpool.tile([P, ND, ck0], io_dt, tag="x")
        nc.sync.dma_start(x0_t[:, :4, :], xT_r[:, :4, slice(0, ck0)])
        nc.scalar.dma_start(x0_t[:, 4:, :], xT_r[:, 4:, slice(0, ck0)])
        win0_t = winpool.tile([P, ND, 512], io_dt, tag="win")
        nc.sync.dma_start(win0_t[:, :4, :], win_r[:, :4, 0:512])
        nc.scalar.dma_start(win0_t[:, 4:, :], win_r[:, 4:, 0:512])

        # b_in is needed by the first gelu; it's tiny — SWDGE queue.
        bin_t = consts.tile([P, NF], dt.float32)
        nc.gpsimd.dma_start(bin_t[:], bin_.ap().rearrange("(fo fi) -> fi fo", fi=P))

        # PE HAM warm-up: ~3us of junk matmuls on a scratch tile while the
        # x0/win0 DMAs are in flight, so real matmuls start at 2.4 GHz
        # instead of spending the first activity window at 1.2 GHz.
        wu_t = consts.tile([P, P], io_dt)
        nc.gpsimd.memset(wu_t[:], 0.0)
        wu_ps = ctx.enter_context(tc.tile_pool(name="wups", bufs=1, space="PSUM"))
        wu_p = wu_ps.tile([P, 64], dt.float32)
        for _ in range(60):
            nc.tensor.matmul(wu_p[:], wu_t[:], wu_t[:, :64], start=True, stop=True)

        # Remaining bulk loads share the Sync HWDGE queue with the W_in
        # stripes, hand-interleaved below so each arrives just in time:
        # the queue drains in emission order, so wout stripe k loads
        # during phase-A stripe k's ~7us of matmuls and the whole of
        # W_out is resident right when phase B first needs it. (Putting
        # them on another queue doesn't work: the scheduler hoists
        # ready DMA triggers, and they'd steal HBM bandwidth from the
        # critical x0/win0 loads.)
        bout_t = consts.tile([P, ND], dt.float32)
        w_t = consts.tile([P, C], dt.float32)
        wout_tiles = []
        if fp16_path:
            for fo in range(8):
                wout_tiles.append(
                    woutpool.tile([P, 4, D], io_dt,
                                  tag=f"wout{fo}", name=f"wout{fo}"))

        off = 0
        for ci, ck in enumerate(chunks):
            csl = slice(off, off + ck)
            x_t = x0_t if ci == 0 else x_dma(ck, csl)

            # ---- phase A: h = gelu(W_in^T @ x + b_in), laid out [f, tok]
            h_t = hpool.tile([P, NF, ck], io_dt, tag="h")
            for fo in range(8):  # 512-wide stripes of F
                win_t = win0_t if (ci == 0 and fo == 0) else win_dma(fo)
                for j in range(4):
                    fc = fo * 4 + j
                    ph = psum_h.tile([P, ck], dt.float32, tag="ph")
                    for dn in range(ND):
                        nc.tensor.matmul(
                            ph[:],
                            win_t[:, dn, j * P:(j + 1) * P],
                            x_t[:, dn, :],
                            start=(dn == 0),
                            stop=(dn == ND - 1),
                        )
                    nc.scalar.activation(
                        h_t[:, fc, :], ph[:],
                        mybir.ActivationFunctionType.Gelu,
                        bias=bin_t[:, fc:fc + 1],
                    )
                if ci == 0:
                    if fp16_path:
                        # interleave the resident W_out load with the
                        # W_in stream: stripe fo rides the queue behind
                        # win stripe fo, loading during its ~7us of
                        # matmuls, so W_out has landed by phase B.
                        nc.sync.dma_start(
                            wout_tiles[fo][:],
                            wout_r[:, fo * 4:(fo + 1) * 4, :])
                    if fo == 3:
                        nc.sync.dma_start(
                            bout_t[:],
                            bout.ap().rearrange("(do di) -> di do", di=P))
                    elif fo == 5:
                        nc.sync.dma_start(w_t[:], wcomb.ap())

            # ---- phase B: y = w * (W_out^T @ h + b_out), laid out [d, tok]
            if fp16_path:
                for dn in range(ND):
                    py = psum_y.tile([P, ck], dt.float32, tag="py")
                    for fc in range(NF):
                        nc.tensor.matmul(
                            py[:],
                            wout_tiles[fc // 4][:, fc % 4, dn * P:(dn + 1) * P],
                            h_t[:, fc, :],
                            start=(fc == 0),
                            stop=(fc == NF - 1),
                        )
                    y_t = ypool.tile([P, ck], dt.float32, tag="y")
                    # one DVE op: (psum + b_out) * w — keeps ScalarE on
                    # gelu only (no ACT table switching per chunk)
                    nc.vector.scalar_tensor_tensor(
                        y_t[:], py[:], bout_t[:, dn:dn + 1], w_t[:, csl],
                        op0=mybir.AluOpType.add, op1=mybir.AluOpType.mult,
                    )
                    nc.scalar.dma_start(yT_r[:, dn, csl], y_t[:])
            else:
                # fp32: W_out too big to keep resident; stream it per chunk
                # in two d-halves (4 PSUM banks live per half).
                for dh in range(2):
                    pys = []
                    for i in range(4):
                        py = psum_y.tile([P, ck], dt.float32, tag=f"py{i}")
                        pys.append(py)
                    for fc in range(NF):
                        wt = woutpool.tile([P, 512], io_dt, tag="wouts")
                        nc.sync.dma_start(
                            wt[:], wout_r[:, fc, dh * 512:(dh + 1) * 512])
                        for i in range(4):
                            nc.tensor.matmul(
                                py := pys[i],
                                wt[:, i * P:(i + 1) * P],
                                h_t[:, fc, :],
                                start=(fc == 0),
                                stop=(fc == NF - 1),
                            )
                    for i in range(4):
                        dn = dh * 4 + i
                        y_t = ypool.tile([P, ck], dt.float32, tag="y")
                        nc.scalar.activation(
                            y_t[:], pys[i][:],
                            mybir.ActivationFunctionType.Identity,
                            bias=bout_t[:, dn:dn + 1],
                        )
                        nc.vector.tensor_mul(y_t[:], y_t[:], w_t[:, csl])
                        nc.sync.dma_start(yT_r[:, dn, csl], y_t[:])
            off += ck

    nc.compile()
    return nc


def _get_nc(C, prec):
    key = (C, prec)
    if key not in _cache:
        _cache[key] = _build_bass(C, prec)
    return _cache[key]


def _route(x, W_router):
    """Host-side router: top-2 selection + renormalized weights (fp64).

    Matches jax.lax.top_k on softmax(logits): softmax is monotone so
    top-2 of logits is identical, with ties broken toward lower index
    (argsort stable on -logits).
    """
    lg = x.astype(np.float64) @ W_router.T.astype(np.float64)
    top2 = np.argsort(-lg, axis=1, kind="stable")[:, :TOP_K]
    l1 = np.take_along_axis(lg, top2[:, 0:1], 1)
    l2 = np.take_along_axis(lg, top2[:, 1:2], 1)
    e2 = np.exp(l2 - l1)
    w1 = (1.0 / (1.0 + e2)).astype(np.float32)
    w2 = (e2 / (1.0 + e2)).astype(np.float32)
    return top2, np.concatenate([w1, w2], axis=1)


def kernel(residual, W_router, W_in, b_in, W_out, b_out):
    global LAST_RESULT
    prec = os.environ.get("MOE_PREC", "fp16")
    np_io = np.float16 if prec != "fp32" else np.float32

    x = np.ascontiguousarray(np.asarray(residual, dtype=np.float32).reshape(T, D))
    W_in = np.asarray(W_in, dtype=np.float32)
    W_out = np.asarray(W_out, dtype=np.float32)
    b_in = np.asarray(b_in, dtype=np.float32)
    b_out = np.asarray(b_out, dtype=np.float32)

    top2, wts = _route(x, np.asarray(W_router, dtype=np.float32))

    idxs, ws = [], []
    for e in range(E):
        sel0 = top2[:, 0] == e
        sel1 = top2[:, 1] == e
        idx = np.concatenate([np.where(sel0)[0], np.where(sel1)[0]])
        w = np.concatenate([wts[sel0, 0], wts[sel1, 1]])
        idxs.append(idx)
        ws.append(w)

    C = max(len(i) for i in idxs)
    C = ((C + P - 1) // P) * P
    nc = _get_nc(C, prec)

    xt = np.ascontiguousarray(x.T)  # [D, T]
    in_maps = []
    for e in range(E):
        cnt = len(idxs[e])
        xT_e = np.zeros((D, C), dtype=np_io)
        xT_e[:, :cnt] = xt[:, idxs[e]]
        wc_e = np.zeros((P, C), dtype=np.float32)
        wc_e[:, :cnt] = ws[e][None, :]
        in_maps.append({
            "xT": xT_e,
            "win": np.ascontiguousarray(W_in[e], dtype=np_io),
            "wout": np.ascontiguousarray(W_out[e], dtype=np_io),
            "bin": b_in[e],
            "bout": b_out[e],
            "wcomb": wc_e,
        })

    if os.environ.get("BASS_TRACE"):
        _install_ntff_hook()
    LAST_RESULT = run_bass_kernel_spmd(nc, in_maps, list(range(NCORES)))

    y = np.zeros((T, D), dtype=np.float32)
    for e in range(E):
        cnt = len(idxs[e])
        y[idxs[e]] += LAST_RESULT.results[e]["yT"][:, :cnt].T
    return y.reshape(B, S, D)



# revision 2
# speedup vs baseline: 1.0298x; 1.0298x over previous
"""MoE MLP (top-2 routing, 8 experts) on 8 Trainium2 NeuronCores.

Expert-parallel: each core owns one expert's weights. The router
(0.05% of FLOPs) runs on the host, which doubles as the dispatch step:
tokens are gathered per selected expert and shipped to that expert's
core. Each core runs a fused gelu-MLP Bass kernel over its routed
tokens in a transposed layout (tokens along the moving axis):

    yT = w * (W_out^T @ gelu(W_in^T @ xT + b_in) + b_out)

Capacity-factor-1 dispatch: the device kernel is compiled for a fixed
per-expert capacity C = T*K/E = 2048 tokens. Expert overflow beyond C
(~137 of 16384 token-expert pairs for the bench routing, 0.8% of
FLOPs) is computed exactly on the host during the scatter-add, like a
capacity-limited MoE that computes (rather than drops) its overflow.
A fixed C also means one cached NEFF regardless of routing.

Performance notes (vs the 500us first version):
- All DRAM tensors are host-packed into the exact per-partition byte
  streams the SBUF tiles need, so every DMA is 128 fat contiguous
  descriptors (near 358 GB/s HBM rate) instead of thousands of 1KB
  strided reads (~150 GB/s).
- Both weight matrices live resident in SBUF (64KB/partition each in
  fp16); W_in is no longer re-streamed per token chunk.
- The first x chunk is split into per-dn 128KB pieces and W_in stripe
  0 into per-f-column 256KB pieces so the first real matmul issues at
  ~1.2us; a dozen junk matmuls bridge t=0 to that point so the PE HAM
  un-throttles (1.2 -> 2.4 GHz) ~4us in, not ~25us.
- The last chunk is the smallest (256 tokens) and y is written back
  in fp16 per-dn pieces as each finishes, cutting the post-compute
  tail from ~11us to ~3us.

Matmuls run in fp16 (4x fp32 PE rate; ~4e-4 end-to-end error vs the
fp32 reference). fp8 was evaluated and rejected: e4m3 DoubleRow
measures ~4.6e-2 end-to-end (tolerance 2e-2), and error-compensated
fp8 needs 3 matmul passes, losing to fp16 outright.
"""

import contextlib
import ctypes
import os
import sys
import types
from contextlib import ExitStack

import numpy as np

import concourse.bass as bass
import concourse.mybir as mybir
import concourse.tile as tile
from concourse import bacc
from concourse.bass_utils import run_bass_kernel_spmd


def _install_ntff_hook():
    """Provide antenv.axon_hooks (absent in this image) so BASS_TRACE=1
    can capture NTFF profiles through the axon PJRT .so. No-op if the
    module already exists or the .so/symbols are unavailable."""
    try:
        from antenv.axon_hooks import get_axon_ntff_profile_hook  # noqa: F401
        return
    except ImportError:
        pass
    so_path = "/opt/axon/libaxon_pjrt.so"
    if not os.path.exists(so_path):
        return
    try:
        lib = ctypes.CDLL(so_path)
    except OSError:
        return
    if not hasattr(lib, "axon_start_nrt_profile"):
        return
    lib.axon_start_nrt_profile.argtypes = [
        ctypes.POINTER(ctypes.c_int64), ctypes.c_size_t]
    lib.axon_start_nrt_profile.restype = ctypes.c_int64
    lib.axon_stop_nrt_profile.argtypes = [ctypes.c_char_p]
    lib.axon_stop_nrt_profile.restype = ctypes.c_int64

    @contextlib.contextmanager
    def _hook(output_dir, device_ids):
        import jax
        jax.devices()  # force PJRT init so the .so's client exists
        if device_ids:
            ids = (ctypes.c_int64 * len(device_ids))(*device_ids)
            rc = lib.axon_start_nrt_profile(ids, len(device_ids))
        else:
            rc = lib.axon_start_nrt_profile(None, 0)
        if rc != 0:
            raise RuntimeError(f"axon_start_nrt_profile rc={rc}")
        try:
            yield
        finally:
            n = lib.axon_stop_nrt_profile(str(output_dir).encode())
            print(f"ntff profile: {n} file(s) -> {output_dir}", file=sys.stderr)

    import antenv
    mod = types.ModuleType("antenv.axon_hooks")
    mod.get_axon_ntff_profile_hook = lambda: _hook
    mod.set_axon_ntff_profile_hook = lambda h: None
    sys.modules["antenv.axon_hooks"] = mod
    antenv.axon_hooks = mod


B, S, D, F, E = 4, 2048, 1024, 4096, 8
T = B * S
TOP_K = 2
NCORES = 8
P = 128
ND, NF = D // P, F // P  # 8, 32
C = 2048                 # device capacity per expert (= T*TOP_K/E)
# PSUM-bank-sized token chunks; smallest last so the drain tail is short.
CHUNKS = [512, 512, 512, 256, 256]
assert sum(CHUNKS) == C

# test.py pokes this for profiling info
LAST_RESULT = None

_cache = {}


def _build_bass():
    dt = mybir.dt
    io = dt.float16
    nc = bacc.Bacc("TRN2", target_bir_lowering=False, debug=False)

    # Host-packed layouts (all 2D [128, cols]; every DMA below slices
    # contiguous columns => 128 fat contiguous descriptors):
    #   x    [P, sum(8*ck)]  col = (chunk | dn | tok):  x[dn*128+p, tok]
    #   win  [P, 8*4*8*128]  col = (fo | j | dn | fi):  W_in[dn*128+p, fo*512+j*128+fi]
    #   wout [P, 8*4*1024]   col = (g | jj | d):        W_out[(g*4+jj)*128+p, d]
    #   y    [P, sum(8*ck)]  col = (chunk | dn | tok):  y[dn*128+p, tok]
    x = nc.dram_tensor("x", [P, ND * C], io, kind="ExternalInput")
    win = nc.dram_tensor("win", [P, NF * P * ND], io, kind="ExternalInput")
    wout = nc.dram_tensor("wout", [P, NF * P * ND], io, kind="ExternalInput")
    bin_ = nc.dram_tensor("bin", [P, NF], dt.float32, kind="ExternalInput")
    bout = nc.dram_tensor("bout", [P, ND], dt.float32, kind="ExternalInput")
    wcomb = nc.dram_tensor("wcomb", [P, C], io, kind="ExternalInput")
    y = nc.dram_tensor("y", [P, ND * C], io, kind="ExternalOutput")

    xa, wina, wouta, ya = x.ap(), win.ap(), wout.ap(), y.ap()

    with tile.TileContext(nc) as tc, ExitStack() as ctx:
        consts = ctx.enter_context(tc.tile_pool(name="consts", bufs=1))
        xpool = ctx.enter_context(tc.tile_pool(name="x", bufs=2))
        wpool = ctx.enter_context(tc.tile_pool(name="w", bufs=1))
        hpool = ctx.enter_context(tc.tile_pool(name="h", bufs=1))
        ypool = ctx.enter_context(tc.tile_pool(name="y", bufs=4))
        psum_h = ctx.enter_context(tc.tile_pool(name="ph", bufs=4, space="PSUM"))
        psum_y = ctx.enter_context(tc.tile_pool(name="py", bufs=2, space="PSUM"))

        win_t = wpool.tile([P, NF // 4, 4, ND, P], io, name="win")
        wout_t = wpool.tile([P, NF // 4, 4, ND * P], io, name="wout")

        # --- head: first x chunk in per-dn pieces + W_in stripe 0 in
        # per-f-column pieces, on the two HWDGE queues, so the first
        # matmul can issue ~1.2us in and its accumulation chain stays
        # just behind the arriving pieces.
        ck0 = CHUNKS[0]
        x0_t = xpool.tile([P, ND, ck0], io, tag="x")
        for dn in range(ND):
            nc.sync.dma_start(x0_t[:, dn, :], xa[:, dn * ck0:(dn + 1) * ck0])
        for j in range(4):
            nc.scalar.dma_start(win_t[:, 0, j, :, :],
                                wina[:, j * 1024:(j + 1) * 1024])
        # remaining W_in stripes whole (1MB each) behind them
        for fo in range(1, 8):
            nc.scalar.dma_start(win_t[:, fo, :, :, :],
                                wina[:, fo * 4096:(fo + 1) * 4096])
        # W_out stripes ride the sync queue behind x chunk 0 — all
        # resident well before phase B of chunk 0 (~56us in).
        for g in range(8):
            nc.sync.dma_start(wout_t[:, g, :, :],
                              wouta[:, g * 4096:(g + 1) * 4096])
        # x chunks 1..4 prefetch behind the W_in stripes on scalar.
        x_tiles = [x0_t]
        off = ck0
        for ck in CHUNKS[1:]:
            x_t = xpool.tile([P, ND, ck], io, tag="x")
            nc.scalar.dma_start(x_t[:], xa[:, ND * off:ND * (off + ck)])
            x_tiles.append(x_t)
            off += ck

        # small constants on the SWDGE queue
        bin_t = consts.tile([P, NF], dt.float32)
        nc.gpsimd.dma_start(bin_t[:], bin_.ap())
        bout_t = consts.tile([P, ND], dt.float32)
        nc.gpsimd.dma_start(bout_t[:], bout.ap())
        w_t = consts.tile([P, C], io)
        nc.gpsimd.dma_start(w_t[:], wcomb.ap())

        # PE HAM bridge: ~1.3us of junk matmuls so the PE busy-window
        # starts accumulating at t=0 and real matmuls continue it.
        wu_t = consts.tile([P, P], io)
        nc.gpsimd.memset(wu_t[:], 0.0)
        wu_ps = ctx.enter_context(tc.tile_pool(name="wups", bufs=1, space="PSUM"))
        wu_p = wu_ps.tile([P, 64], dt.float32)
        for _ in range(12):
            nc.tensor.matmul(wu_p[:], wu_t[:], wu_t[:, :64], start=True, stop=True)

        off = 0
        for ci, ck in enumerate(CHUNKS):
            x_t = x_tiles[ci]
            # ---- phase A: h = gelu(W_in^T @ x + b_in), laid out [f, tok]
            h_t = hpool.tile([P, NF, ck], io, tag="h")
            for fc in range(NF):
                fo, j = fc // 4, fc % 4
                ph = psum_h.tile([P, ck], dt.float32, tag="ph")
                for dn in range(ND):
                    nc.tensor.matmul(
                        ph[:],
                        win_t[:, fo, j, dn, :],
                        x_t[:, dn, :],
                        start=(dn == 0),
                        stop=(dn == ND - 1),
                    )
                nc.scalar.activation(
                    h_t[:, fc, :], ph[:],
                    mybir.ActivationFunctionType.Gelu,
                    bias=bin_t[:, fc:fc + 1],
                )
            # ---- phase B: y = w * (W_out^T @ h + b_out), laid out [d, tok]
            for dn in range(ND):
                py = psum_y.tile([P, ck], dt.float32, tag="py")
                for fc in range(NF):
                    nc.tensor.matmul(
                        py[:],
                        wout_t[:, fc // 4, fc % 4, dn * P:(dn + 1) * P],
                        h_t[:, fc, :],
                        start=(fc == 0),
                        stop=(fc == NF - 1),
                    )
                y_t = ypool.tile([P, ck], io, tag="y")
                # one DVE op: (psum + b_out) * w — keeps ScalarE on
                # gelu only (no ACT table switching per chunk)
                nc.vector.scalar_tensor_tensor(
                    y_t[:], py[:], bout_t[:, dn:dn + 1], w_t[:, off:off + ck],
                    op0=mybir.AluOpType.add, op1=mybir.AluOpType.mult,
                )
                nc.sync.dma_start(
                    ya[:, ND * off + dn * ck:ND * off + (dn + 1) * ck], y_t[:])
            off += ck

    nc.compile()
    return nc


def _get_nc():
    if "nc" not in _cache:
        _cache["nc"] = _build_bass()
    return _cache["nc"]


def _route(x, W_router):
    """Host-side router: top-2 selection + renormalized weights (fp64).

    Matches jax.lax.top_k on softmax(logits): softmax is monotone so
    top-2 of logits is identical, with ties broken toward lower index
    (argsort stable on -logits).
    """
    lg = x.astype(np.float64) @ W_router.T.astype(np.float64)
    top2 = np.argsort(-lg, axis=1, kind="stable")[:, :TOP_K]
    l1 = np.take_along_axis(lg, top2[:, 0:1], 1)
    l2 = np.take_along_axis(lg, top2[:, 1:2], 1)
    e2 = np.exp(l2 - l1)
    w1 = (1.0 / (1.0 + e2)).astype(np.float32)
    w2 = (e2 / (1.0 + e2)).astype(np.float32)
    return top2, np.concatenate([w1, w2], axis=1)


def _gelu(v):
    try:
        from scipy.special import erf
    except ImportError:
        import math
        erf = np.vectorize(math.erf, otypes=[np.float64])
    return 0.5 * v * (1.0 + erf(v / np.sqrt(2.0)))


def kernel(residual, W_router, W_in, b_in, W_out, b_out):
    global LAST_RESULT

    x = np.ascontiguousarray(np.asarray(residual, dtype=np.float32).reshape(T, D))
    W_in = np.asarray(W_in, dtype=np.float32)
    W_out = np.asarray(W_out, dtype=np.float32)
    b_in = np.asarray(b_in, dtype=np.float32)
    b_out = np.asarray(b_out, dtype=np.float32)

    top2, wts = _route(x, np.asarray(W_router, dtype=np.float32))

    idxs, ws = [], []
    for e in range(E):
        sel0 = top2[:, 0] == e
        sel1 = top2[:, 1] == e
        idx = np.concatenate([np.where(sel0)[0], np.where(sel1)[0]])
        w = np.concatenate([wts[sel0, 0], wts[sel1, 1]])
        idxs.append(idx)
        ws.append(w)

    nc = _get_nc()

    xt = np.ascontiguousarray(x.T).astype(np.float16)  # [D, T]
    in_maps = []
    for e in range(E):
        dev_idx = idxs[e][:C]
        cnt = len(dev_idx)
        xsel = np.zeros((D, C), dtype=np.float16)
        xsel[:, :cnt] = xt[:, dev_idx]
        xsel = xsel.reshape(ND, P, C)
        xp = np.empty((P, ND * C), dtype=np.float16)
        o = 0
        for ck in CHUNKS:
            xp[:, ND * o:ND * (o + ck)] = (
                xsel[:, :, o:o + ck].transpose(1, 0, 2).reshape(P, ND * ck))
            o += ck
        wc = np.zeros(C, dtype=np.float16)
        wc[:cnt] = ws[e][:cnt]
        in_maps.append({
            "x": xp,
            "win": np.ascontiguousarray(
                W_in[e].astype(np.float16).reshape(ND, P, 8, 4, P)
                .transpose(1, 2, 3, 0, 4).reshape(P, -1)),
            "wout": np.ascontiguousarray(
                W_out[e].astype(np.float16).reshape(8, 4, P, D)
                .transpose(2, 0, 1, 3).reshape(P, -1)),
            "bin": np.ascontiguousarray(b_in[e].reshape(NF, P).T),
            "bout": np.ascontiguousarray(b_out[e].reshape(ND, P).T),
            "wcomb": np.ascontiguousarray(
                np.broadcast_to(wc[None, :], (P, C))),
        })

    if os.environ.get("BASS_TRACE"):
        _install_ntff_hook()
    LAST_RESULT = run_bass_kernel_spmd(nc, in_maps, list(range(NCORES)))

    y = np.zeros((T, D), dtype=np.float32)
    for e in range(E):
        dev_idx = idxs[e][:C]
        cnt = len(dev_idx)
        yp = LAST_RESULT.results[e]["y"]  # [P, ND*C] fp16
        o = 0
        for ck in CHUNKS:
            lo, hi = o, min(o + ck, cnt)
            if lo >= cnt:
                break
            blk = (yp[:, ND * o:ND * (o + ck)].reshape(P, ND, ck)
                   .transpose(1, 0, 2).reshape(D, ck))
            y[dev_idx[lo:hi]] += blk[:, :hi - lo].T.astype(np.float32)
            o += ck
        # capacity overflow: computed exactly on the host (fp32)
        if len(idxs[e]) > C:
            oidx = idxs[e][C:]
            ow = ws[e][C:].astype(np.float32)
            h = _gelu(x[oidx] @ W_in[e] + b_in[e]).astype(np.float32)
            y[oidx] += ow[:, None] * (h @ W_out[e] + b_out[e])
    return y.reshape(B, S, D)
